# revision 1
# baseline (speedup 1.0000x reference)
"""Distributed Trainium2 Bass kernel for BrosAttention.

B=2, S=1024, H=768, NH=12, DH=64:
  q,k,v = heads(hidden @ W.T + b)
  scores = q@k^T + einsum('bnid,bijd->bnij', q, bpe)   (bpe = bbox transposed)
  probs  = softmax(scores / 8)
  out    = LN(probs@v @ Wo.T + bo + hidden)

Sharding: 8 cores = 2 batches x 4 query-row blocks of 256 rows. Each core
reads only its 64MB slice of bbox_pos_emb, computes K/V for the full
sequence of its batch (duplicated 4x, cheaper than a collective here), and
writes a disjoint [256, 768] output slice. No collectives.

Layout: transposed scores (scoresT[j, i] per head) because the bias term
q.bpe needs d on partitions; bpe arrives [j, d] and is PE-transposed with
two query rows packed per [128, j] tile. The bias matmul (lhsT = q of one
row as a [64, 12] weight) runs 4 i's concurrently in the four 32-column
groups of the PE array; bias tiles are PE-transposed again into [j, (i,n)]
and added to QK^T psum tiles via a stride-12 AP. Softmax-over-partitions
uses ones-vector matmuls; probs stay unnormalized until after P@V.
"""

import os
import sys
import numpy as np

sys.path.insert(0, "/opt/trn_rl_repo")

B, S, H, NH, DH = 2, 1024, 768, 12, 64
EPS = 1e-12
P = 128
I_CORE = S * B // 8  # 256
N_CORES = 8

_COMPILED = {}


def build_kernel(s=S, i_core=I_CORE, h=H, nh=NH, dh=DH):
    from contextlib import ExitStack
    from concourse import bacc, bass, mybir, tile

    f32 = mybir.dt.float32
    bf16 = mybir.dt.bfloat16
    Alu = mybir.AluOpType
    Act = mybir.ActivationFunctionType
    AxisX = mybir.AxisListType.X

    SC = s // P          # 8 seq chunks
    HC = h // P          # 6 hidden chunks
    IH = i_core // 2     # 128 i's per half
    NDUO_H = IH // 4     # 32 duos per half
    JH = min(512, s)     # fp32 matmul N limit / psum bank
    NJH = s // JH        # 2
    HP = nh // 2         # 6 head pairs
    VH = h // 2          # 384

    nc = bacc.Bacc(None, target_bir_lowering=False, debug=False)

    bf16_ = mybir.dt.bfloat16
    d_hidT = nc.declare_dram_parameter("hidT", [HC, P, s], bf16_, isOutput=False)
    d_hidRT = nc.declare_dram_parameter("hidRT", [HC, P, i_core], bf16_, isOutput=False)
    d_hidR = nc.declare_dram_parameter("hid_rows", [i_core // P, P, h], f32, isOutput=False)
    d_bpe = nc.declare_dram_parameter("bpe", [i_core, dh, s], bf16_, isOutput=False)
    d_W = {w: nc.declare_dram_parameter(w + "T", [HC, P, h], bf16_, isOutput=False)
           for w in ("Wq", "Wk", "Wv", "Wo")}
    d_b = {bn: nc.declare_dram_parameter(bn, [1, h], f32, isOutput=False)
           for bn in ("bq", "bk", "bv", "bo", "ln_gamma", "ln_beta")}
    d_ident = nc.declare_dram_parameter("ident", [P, P], f32, isOutput=False)
    d_out = nc.declare_dram_parameter("out", [i_core // P, P, h], f32, isOutput=True)

    with tile.TileContext(nc) as tc, ExitStack() as ctx:
        # ------------- long-lived pools -------------
        const_p = ctx.enter_context(tc.tile_pool(name="const", bufs=1))
        stat_p = ctx.enter_context(tc.tile_pool(name="stat", bufs=1))
        ps128 = ctx.enter_context(
            tc.tile_pool(name="ps128", bufs=3, space=bass.MemorySpace.PSUM))
        ps512 = ctx.enter_context(
            tc.tile_pool(name="ps512", bufs=1, space=bass.MemorySpace.PSUM))
        psB = ctx.enter_context(
            tc.tile_pool(name="psB", bufs=2, space=bass.MemorySpace.PSUM))
        psS = ctx.enter_context(
            tc.tile_pool(name="psS", bufs=1, space=bass.MemorySpace.PSUM))
        psC = ctx.enter_context(
            tc.tile_pool(name="psC", bufs=1, space=bass.MemorySpace.PSUM))

        # ------------- constants -------------
        ident = const_p.tile([P, P], f32)
        nc.sync.dma_start(ident[:], d_ident[:])
        ones_col = const_p.tile([P, 1], f32)
        nc.vector.memset(ones_col[:], 1.0)
        ones_row = const_p.tile([1, s], f32)
        nc.vector.memset(ones_row[:], 1.0)
        eps_t = const_p.tile([P, 1], f32)
        nc.vector.memset(eps_t[:], EPS)
        zrow = const_p.tile([1, P], bf16)
        nc.vector.memset(zrow[:], 0.0)
        ident_bf = const_p.tile([P, P], bf16)
        nc.vector.tensor_copy(ident_bf[:], ident[:])
        ones_col_bf = const_p.tile([P, 1], bf16)
        nc.vector.memset(ones_col_bf[:], 1.0)
        ones_row_bf = const_p.tile([1, s], bf16)
        nc.vector.memset(ones_row_bf[:], 1.0)
        b_sb = {}
        b_bf = {}
        for bn in ("bq", "bk", "bv", "bo", "ln_gamma", "ln_beta"):
            b_sb[bn] = const_p.tile([1, h], f32, name=f"bias_{bn}")
            nc.sync.dma_start(b_sb[bn][:], d_b[bn][:])
            b_bf[bn] = const_p.tile([1, h], bf16, name=f"biasbf_{bn}")
            nc.vector.tensor_copy(b_bf[bn][:], b_sb[bn][:])

        bcast = {}
        for bn in ("ln_gamma", "ln_beta"):
            t = stat_p.tile([P, h], f32, name=f"bcast_{bn}")
            for c in range(HC):
                pbx = ps128.tile([P, P], f32, name="pt")
                nc.tensor.matmul(pbx[:], ones_row[:, 0:P],
                                 b_sb[bn][:, c * P:(c + 1) * P])
                nc.scalar.copy(t[:, c * P:(c + 1) * P], pbx[:])
            bcast[bn] = t

        # long-lived activations
        hidR = stat_p.tile([P, i_core // P, h], f32)
        nc.sync.dma_start(hidR[:], d_hidR[:].transpose([1, 0, 2]))
        WoT = stat_p.tile([P, HC, h], bf16)
        nc.sync.dma_start(WoT[:], d_W["Wo"][:].transpose([1, 0, 2]))
        qT128 = stat_p.tile([P, nh, i_core], bf16)  # q[n,i,:] at both 64-halves
        qPair = stat_p.tile([P, i_core // 2, 32], bf16)
        kT128 = stat_p.tile([P, HP, s], bf16)
        v_sb = stat_p.tile([P, SC, h], bf16)

        def pe_T(dst_ap, src_ap, copy_eng):
            bf = src_ap.dtype == bf16
            pt = ps128.tile([P, P], bf16 if bf else f32, name="pt")
            n = src_ap.shape[-1]
            nc.tensor.transpose(pt[0:n, :], src_ap,
                                ident_bf[:] if bf else ident[:])
            if copy_eng is nc.scalar:
                copy_eng.copy(dst_ap, pt[0:n, :])
            else:
                copy_eng.tensor_copy(dst_ap, pt[0:n, :])

        # ------------- phase 0 -------------
        with tc.tile_pool(name="early", bufs=1) as early_p:
            hidT = early_p.tile([P, HC, s], bf16)
            nc.sync.dma_start(hidT[:], d_hidT[:].transpose([1, 0, 2]))
            hidRT = early_p.tile([P, HC, i_core], bf16)
            nc.sync.dma_start(hidRT[:], d_hidRT[:].transpose([1, 0, 2]))

            def load_WT(w, dst):
                nc.sync.dma_start(dst[:], d_W[w][:].transpose([1, 0, 2]))
                return dst

            # Q projection (transposed): qT = Wq @ hidR^T + bq
            WqT = load_WT("Wq", early_p.tile([P, HC, h], bf16, name="WT_q"))
            for r in range(HC):
                pq = ps512.tile([P, JH], f32, name="big")
                for kc in range(HC):
                    nc.tensor.matmul(pq[:, 0:i_core],
                                     WqT[:, kc, r * P:(r + 1) * P],
                                     hidRT[:, kc, :], start=(kc == 0), stop=False)
                nc.tensor.matmul(pq[:, 0:i_core], b_bf["bq"][:, r * P:(r + 1) * P],
                                 ones_row_bf[:, 0:i_core], start=False, stop=True)
                for sub in range(2):
                    src = pq[sub * dh:(sub + 1) * dh, 0:i_core]
                    nc.vector.tensor_copy(qT128[0:dh, 2 * r + sub, :], src)
                    nc.vector.tensor_copy(qT128[dh:P, 2 * r + sub, :], src)

            # qPair[k, p, m]: block-diag bias weights: rows 0-63 =
            # q_{2p} in cols 0:12, rows 64-127 = q_{2p+1} in cols 12:24.
            nc.vector.memset(qPair[:], 0.0)
            nc.vector.tensor_copy(
                qPair[0:dh, :, 0:nh],
                qT128[0:dh, :, 0::2].transpose([0, 2, 1]))
            nc.vector.tensor_copy(
                qPair[dh:P, :, nh:2 * nh],
                qT128[dh:P, :, 1::2].transpose([0, 2, 1]))

            # K projection (transposed): kT = Wk @ hid^T + bk
            WkT = load_WT("Wk", early_p.tile([P, HC, h], bf16, name="WT_k"))
            for r in range(HC):
                for jh in range(NJH):
                    pk = ps512.tile([P, JH], f32, name="big")
                    for kc in range(HC):
                        nc.tensor.matmul(pk[:], WkT[:, kc, r * P:(r + 1) * P],
                                         hidT[:, kc, jh * JH:(jh + 1) * JH],
                                         start=(kc == 0), stop=False)
                    nc.tensor.matmul(pk[:], b_bf["bk"][:, r * P:(r + 1) * P],
                                     ones_row_bf[:, 0:JH], start=False, stop=True)
                    nc.vector.tensor_copy(
                        kT128[:, r, jh * JH:(jh + 1) * JH], pk[:])

            # V projection (natural): v = hid @ Wv^T + bv
            WvT = load_WT("Wv", early_p.tile([P, HC, h], bf16, name="WT_v"))
            for jc in range(SC):
                for vh in range(2):
                    pv = ps512.tile([P, JH], f32, name="big")
                    for kc in range(HC):
                        nc.tensor.matmul(pv[:, 0:VH],
                                         hidT[:, kc, jc * P:(jc + 1) * P],
                                         WvT[:, kc, vh * VH:(vh + 1) * VH],
                                         start=(kc == 0), stop=False)
                    nc.tensor.matmul(pv[:, 0:VH], ones_row_bf[:, 0:P],
                                     b_bf["bv"][:, vh * VH:(vh + 1) * VH],
                                     start=False, stop=True)
                    nc.vector.tensor_copy(v_sb[:, jc, vh * VH:(vh + 1) * VH],
                                          pv[:, 0:VH])


        # ------------- phases A+B -------------
        with tc.tile_pool(name="bpeT", bufs=4) as bpeT_p, \
             tc.tile_pool(name="bias4", bufs=1) as bias4_p, \
             tc.tile_pool(name="biasT", bufs=1) as biasT_p, \
             tc.tile_pool(name="sm", bufs=2) as sm_p, \
             tc.tile_pool(name="ctxp", bufs=1) as ctx_p, \
             tc.tile_pool(name="yp", bufs=1) as y_p:
            for half in range(2):
                i0h = half * IH
                # biasT[j, jc, duo*48 + 12*i4 + n]
                biasT = biasT_p.tile([P, SC, NDUO_H * 4 * nh], bf16)

                for octo in range(NDUO_H // 2):
                    pb_h = [psB.tile([P, JH], f32, name="pbh") for j in range(NJH)]
                    for c4 in range(4):
                        pair = octo * 4 + c4
                        iA = i0h + 2 * pair
                        bpeT = bpeT_p.tile([P, s], bf16)
                        nc.sync.dma_start(
                            bpeT[:], d_bpe[iA:iA + 2].rearrange("a b c -> (a b) c"))
                        lhs = qPair[:, (i0h // 2) + pair, 0:32]
                        for jh in range(NJH):
                            nc.tensor.matmul(
                                pb_h[jh][32 * c4:32 * c4 + 32, :], lhs,
                                bpeT[:, jh * JH:(jh + 1) * JH],
                                tile_position=(0, 32 * c4))
                    b4 = bias4_p.tile([P, s], bf16)
                    for jh in range(NJH):
                        nc.vector.tensor_copy(b4[:, jh * JH:(jh + 1) * JH],
                                              pb_h[jh][:])
                    for jc in range(SC):
                        ptb = ps128.tile([P, P], bf16, name="pt")
                        nc.tensor.transpose(ptb[:], b4[:, jc * P:(jc + 1) * P],
                                            ident_bf[:])
                        nc.vector.tensor_copy(
                            biasT[:, jc, octo * 8 * nh:(octo + 1) * 8 * nh
                                  ].rearrange("p (a b) -> p a b", a=4),
                            ptb[:].rearrange("p (a b) -> p a b", a=4)[:, :, 0:2 * nh])

                # ---- attention ----
                ctxT = ctx_p.tile([P, HP, IH], bf16)
                for hp in range(HP):
                    pctx = psC.tile([P, IH], f32, name="pctx")
                    for sub in range(2):
                        n = 2 * hp + sub
                        probsT = sm_p.tile([P, SC, IH], bf16)
                        psum_s = psS.tile([1, IH], f32)
                        for jc in range(SC):
                            pqk = ps128.tile([P, IH], f32, name="pt")
                            sb = sub * dh
                            nc.tensor.matmul(pqk[:],
                                             kT128[sb:sb + dh, hp, jc * P:(jc + 1) * P],
                                             qT128[sb:sb + dh, n, i0h:i0h + IH])
                            sE = sm_p.tile([P, IH], f32)
                            nc.vector.tensor_tensor(
                                sE[:], pqk[:],
                                biasT[:, jc, n:n + nh * (IH - 1) + 1:nh], Alu.add)
                            nc.scalar.activation(probsT[:, jc, :], sE[:],
                                                 Act.Exp, scale=0.125)
                            nc.tensor.matmul(psum_s[:], ones_col_bf[:],
                                             probsT[:, jc, :],
                                             start=(jc == 0), stop=(jc == SC - 1),
                                             skip_group_check=True)
                        rec = sm_p.tile([1, IH], f32)
                        nc.vector.reciprocal(rec[:], psum_s[:])
                        prec = ps128.tile([P, IH], f32, name="pt")
                        nc.tensor.matmul(prec[0:dh, :], ones_row[:, 0:dh], rec[:])
                        recB = sm_p.tile([dh, IH], f32)
                        nc.scalar.copy(recB[:], prec[0:dh, :])
                        for jc in range(SC):
                            nc.tensor.matmul(
                                pctx[sub * dh:(sub + 1) * dh, :],
                                v_sb[:, jc, n * dh:(n + 1) * dh],
                                probsT[:, jc, :],
                                start=(jc == 0), stop=(jc == SC - 1),
                                tile_position=(0, sub * dh),
                                skip_group_check=True)
                        nc.vector.tensor_tensor(
                            pctx[sub * dh:(sub + 1) * dh, :],
                            pctx[sub * dh:(sub + 1) * dh, :],
                            recB[:], Alu.mult)
                    nc.scalar.copy(ctxT[:, hp, :], pctx[:])

                # ---- O-proj + residual + LN ----
                pys = [ps512.tile([P, VH], f32, name="big") for j in range(2)]
                for vh in range(2):
                    for kc in range(HC):
                        nc.tensor.matmul(pys[vh][:], ctxT[:, kc, :],
                                         WoT[:, kc, vh * VH:(vh + 1) * VH],
                                         start=(kc == 0), stop=False)
                    nc.tensor.matmul(pys[vh][:], ones_row_bf[:, 0:P],
                                     b_bf["bo"][:, vh * VH:(vh + 1) * VH],
                                     start=False, stop=True)
                y = y_p.tile([P, h], f32)
                for vh in range(2):
                    nc.vector.tensor_tensor(y[:, vh * VH:(vh + 1) * VH],
                                            pys[vh][:],
                                            hidR[:, half, vh * VH:(vh + 1) * VH],
                                            Alu.add)
                mu = y_p.tile([P, 1], f32)
                nc.vector.tensor_reduce(mu[:], y[:], AxisX, Alu.add)
                nc.vector.tensor_scalar(mu[:], mu[:], 1.0 / h, None, Alu.mult)
                yc = y_p.tile([P, h], f32)
                nc.vector.tensor_scalar(yc[:], y[:], mu[:], None, Alu.subtract)
                ssq = y_p.tile([P, 1], f32)
                nc.scalar.activation(y[:], yc[:], Act.Square, accum_out=ssq[:])
                std = y_p.tile([P, 1], f32)
                nc.scalar.activation(std[:], ssq[:], Act.Sqrt,
                                     scale=1.0 / h, bias=eps_t[:])
                rstd = y_p.tile([P, 1], f32)
                nc.vector.reciprocal(rstd[:], std[:])
                o1 = y_p.tile([P, h], f32)
                nc.vector.tensor_scalar(o1[:], yc[:], rstd[:], None, Alu.mult)
                nc.vector.tensor_tensor(o1[:], o1[:], bcast["ln_gamma"][:], Alu.mult)
                nc.vector.tensor_tensor(o1[:], o1[:], bcast["ln_beta"][:], Alu.add)
                nc.sync.dma_start(d_out[half], o1[:])

    nc.compile()
    return nc


def _shard_inputs(inputs):
    import ml_dtypes
    bf = ml_dtypes.bfloat16
    hs = np.ascontiguousarray(np.asarray(inputs["hidden_states"]), dtype=np.float32)
    bpe = np.asarray(inputs["bbox_pos_emb"])
    ident = np.eye(P, dtype=np.float32)
    # per-batch transposed hidden [H, S] in bf16
    hsT = {b: np.ascontiguousarray(hs[b].T.astype(bf)).reshape(H // P, P, S)
           for b in range(B)}
    WT = {w: np.ascontiguousarray(
             np.asarray(inputs[w], dtype=np.float32).T.astype(bf)).reshape(
                 H // P, P, H)
          for w in ("Wq", "Wk", "Wv", "Wo")}
    in_maps = []
    for c in range(N_CORES):
        b = c // 4
        q0 = (c % 4) * I_CORE
        m = {
            "hidT": hsT[b],
            "hidRT": np.ascontiguousarray(
                hs[b, q0:q0 + I_CORE].T.astype(bf)).reshape(H // P, P, I_CORE),
            "hid_rows": np.ascontiguousarray(
                hs[b, q0:q0 + I_CORE].reshape(I_CORE // P, P, H)),
            "bpe": np.ascontiguousarray(
                bpe[q0:q0 + I_CORE, :, b, :].transpose(0, 2, 1).astype(bf)),
            "ident": ident,
        }
        for w in ("Wq", "Wk", "Wv", "Wo"):
            m[w + "T"] = WT[w]
        for bn in ("bq", "bk", "bv", "bo", "ln_gamma", "ln_beta"):
            m[bn] = np.ascontiguousarray(
                np.asarray(inputs[bn], dtype=np.float32).reshape(1, H))
        in_maps.append(m)
    return in_maps


def _install_ntff_shim():
    """The agent image's antenv lacks axon_hooks; recreate the NTFF profile
    hook via ctypes against libaxon_pjrt.so so trace=True yields
    exec_time_ns + a perfetto trace."""
    import sys as _sys
    if "antenv.axon_hooks" in _sys.modules:
        return
    import types, ctypes, contextlib
    so_path = "/opt/axon/libaxon_pjrt.so"
    mod = types.ModuleType("antenv.axon_hooks")
    _state = {}

    def get_axon_ntff_profile_hook():
        if "hook" in _state:
            return _state["hook"]
        try:
            lib = ctypes.CDLL(so_path)
            if not hasattr(lib, "axon_start_nrt_profile"):
                _state["hook"] = None
                return None
            lib.axon_start_nrt_profile.argtypes = [
                ctypes.POINTER(ctypes.c_int64), ctypes.c_size_t]
            lib.axon_start_nrt_profile.restype = ctypes.c_int64
            lib.axon_stop_nrt_profile.argtypes = [ctypes.c_char_p]
            lib.axon_stop_nrt_profile.restype = ctypes.c_int64
        except OSError:
            _state["hook"] = None
            return None

        @contextlib.contextmanager
        def _hook(output_dir, device_ids):
            import jax
            jax.devices()
            if device_ids:
                ids = (ctypes.c_int64 * len(device_ids))(*device_ids)
                rc = lib.axon_start_nrt_profile(ids, len(device_ids))
            else:
                rc = lib.axon_start_nrt_profile(None, 0)
            if rc != 0:
                raise RuntimeError(f"axon_start_nrt_profile rc={rc}")
            try:
                yield
            finally:
                n = lib.axon_stop_nrt_profile(str(output_dir).encode())
                print(f"ntff profile: {n} file(s) written to {output_dir}")

        _state["hook"] = _hook
        return _hook

    mod.get_axon_ntff_profile_hook = get_axon_ntff_profile_hook
    _sys.modules["antenv.axon_hooks"] = mod


def kernel(**inputs):
    from concourse.bass_utils import run_bass_kernel_spmd

    if os.environ.get("BASS_KERNEL_TRACE"):
        _install_ntff_shim()
        import concourse.bass_utils as _bu
        _bu.upload_artifacts = lambda tmpdir: f"file://{tmpdir}"

    if "nc" not in _COMPILED:
        _COMPILED["nc"] = build_kernel()
    nc = _COMPILED["nc"]
    in_maps = _shard_inputs(inputs)
    res = run_bass_kernel_spmd(nc, in_maps, core_ids=list(range(N_CORES)),
                               trace=bool(os.environ.get("BASS_KERNEL_TRACE")))
    _COMPILED["last_result"] = res
    out = np.zeros((B, S, H), dtype=np.float32)
    for c in range(N_CORES):
        b = c // 4
        q0 = (c % 4) * I_CORE
        out[b, q0:q0 + I_CORE] = np.asarray(
            res.results[c]["out"]).reshape(I_CORE, H)
    return out



# revision 23
# speedup vs baseline: 1.5806x; 1.5806x over previous
"""Distributed Trainium2 Bass kernel for BrosAttention.

B=2, S=1024, H=768, NH=12, DH=64:
  q,k,v = heads(hidden @ W.T + b)
  scores = q@k^T + einsum('bnid,bijd->bnij', q, bpe)   (bpe = bbox transposed)
  probs  = softmax(scores / 8)
  out    = LN(probs@v @ Wo.T + bo + hidden)

Sharding: 8 cores = 2 batches x 4 query-row blocks of 256 rows; each core
reads only its slice of bbox_pos_emb (fp8 e3m4, 16.8MB) and writes a
disjoint [256, 768] output block. No collectives.

Structure:
 - fp8 (e3m4) inputs for projections + bias einsum; weights pre-scaled x16
   on host, descaled inside the PSUM-copy activations.
 - Bias einsum: block-diag qPair weights [128,32] (2 query rows x 12 heads,
   n-major columns), 4 pairs concurrent in PE column strips; strips
   transposed back through a host-built permutation matrix so the result
   comes out n-grouped -> the scores+bias add is one contiguous
   tensor_tensor (in-place in PSUM) per [j-chunk, 6-head group].
 - kT/qT stored as 64-partition tiles: every QK matmul reads partition
   base 0 (base-64 operands + offset PSUM writes crash the HW).
 - Softmax sums folded into P@V as a 65th all-ones column of V; PV is
   interleaved with QK per j-chunk, accumulating into a persistent
   [65, 12, 128] psum tile.
"""

import os
import sys
import numpy as np

sys.path.insert(0, "/opt/trn_rl_repo")

B, S, H, NH, DH = 2, 1024, 768, 12, 64
EPS = 1e-12
P = 128
I_CORE = S * B // 8  # 256
N_CORES = 8
WSCALE = 16.0

_COMPILED = {}


def build_kernel():
    from contextlib import ExitStack
    from concourse import bacc, bass, mybir, tile

    f32 = mybir.dt.float32
    bf16 = mybir.dt.bfloat16
    f8 = mybir.dt.float8e3
    Alu = mybir.AluOpType
    Act = mybir.ActivationFunctionType
    AxisX = mybir.AxisListType.X

    SC = S // P            # 8 j chunks
    HC = H // P            # 6 hidden chunks
    IH = I_CORE // 2       # 128 i per half
    NPAIR = I_CORE // 2    # 128 i-pairs per core
    NOCT = 16              # octos (8 i's) per half
    NGRAN = 8              # bpe granule = 8 pairs (1.05 MB DMA)
    HP = NH // 2
    VH = H // 2            # 384

    nc = bacc.Bacc(None, target_bir_lowering=False, debug=False)

    d_hidT8 = nc.declare_dram_parameter("hidT8", [HC, P, S], f8, isOutput=False)
    d_hidRT8 = nc.declare_dram_parameter("hidRT8", [HC, P, I_CORE], f8, isOutput=False)
    d_hidR = nc.declare_dram_parameter("hidR", [2, P, H], f32, isOutput=False)
    d_bpe = nc.declare_dram_parameter("bpe8", [P, NPAIR, S], f8, isOutput=False)
    d_W8 = {w: nc.declare_dram_parameter(w + "8", [HC, P, H], f8, isOutput=False)
            for w in ("Wq", "Wk", "Wv")}
    d_WoT = nc.declare_dram_parameter("WoT", [HC, P, H], bf16, isOutput=False)
    d_bqcol = nc.declare_dram_parameter("bqcol", [P, HC], f32, isOutput=False)
    d_bqcol4 = nc.declare_dram_parameter("bqcol4", [P, HC], f32, isOutput=False)
    d_bkcol = nc.declare_dram_parameter("bkcol", [P, HC], f32, isOutput=False)
    d_bv16 = nc.declare_dram_parameter("bv16", [1, H], bf16, isOutput=False)
    d_bo = nc.declare_dram_parameter("bo_bf", [1, H], bf16, isOutput=False)
    d_gamma = nc.declare_dram_parameter("gamma_bf", [1, H], bf16, isOutput=False)
    d_beta = nc.declare_dram_parameter("beta_bf", [1, H], bf16, isOutput=False)
    d_perm = nc.declare_dram_parameter("perm_bf", [P, P], bf16, isOutput=False)
    d_out = nc.declare_dram_parameter("out", [2, P, H], f32, isOutput=True)

    with tile.TileContext(nc) as tc, ExitStack() as ctx:
        const_p = ctx.enter_context(tc.tile_pool(name="const", bufs=1))
        stat_p = ctx.enter_context(tc.tile_pool(name="stat", bufs=1))
        bpe_p = ctx.enter_context(tc.tile_pool(name="bpe", bufs=3))
        biasT_p = ctx.enter_context(tc.tile_pool(name="biasT", bufs=1))
        b4_p = ctx.enter_context(tc.tile_pool(name="b4", bufs=2))
        probs_p = ctx.enter_context(tc.tile_pool(name="probs", bufs=3))
        y_p = ctx.enter_context(tc.tile_pool(name="y", bufs=1))

        # ---------------- constants ----------------
        perm_bf = const_p.tile([P, P], bf16)
        nc.sync.dma_start(perm_bf[:], d_perm[:])
        ones_row = const_p.tile([1, P], bf16)
        nc.vector.memset(ones_row[:], 1.0)
        eps_t = const_p.tile([P, 1], f32)
        nc.vector.memset(eps_t[:], EPS)
        bqcol = const_p.tile([P, HC], f32)
        nc.sync.dma_start(bqcol[:], d_bqcol[:])
        bqcol4 = const_p.tile([P, HC], f32)
        nc.sync.dma_start(bqcol4[:], d_bqcol4[:])
        bkcol = const_p.tile([P, HC], f32)
        nc.sync.dma_start(bkcol[:], d_bkcol[:])
        bv16 = const_p.tile([1, H], bf16)
        nc.sync.dma_start(bv16[:], d_bv16[:])
        bo_bf = const_p.tile([1, H], bf16)
        nc.sync.dma_start(bo_bf[:], d_bo[:])
        gamma_r = const_p.tile([1, H], bf16)
        nc.sync.dma_start(gamma_r[:], d_gamma[:])
        beta_r = const_p.tile([1, H], bf16)
        nc.sync.dma_start(beta_r[:], d_beta[:])

        # long-lived activations (kT/qT: 64-partition tiles, base-0 reads)
        kT = stat_p.tile([DH, NH, S], bf16)
        v_sb = stat_p.tile([P, SC, NH, DH + 1], bf16)
        qT = stat_p.tile([DH, NH, I_CORE], bf16)
        qPair8 = stat_p.tile([P, NPAIR, 32], f8)    # block-diag bias weights
        nc.vector.memset(qPair8[:], 0.0)
        hidR = stat_p.tile([P, 2, H], f32)
        nc.sync.dma_start(hidR[:], d_hidR[:].transpose([1, 0, 2]))
        WoT = stat_p.tile([P, HC, H], bf16)
        nc.sync.dma_start(WoT[:], d_WoT[:].transpose([1, 0, 2]))
        gammaB = stat_p.tile([P, H], bf16)
        betaB = stat_p.tile([P, H], bf16)
        ctxT = stat_p.tile([P, HC, IH], bf16)

        # bpe granule streaming
        bpe_tiles = {}

        def fetch_gran(g):
            t = bpe_p.tile([P, NGRAN, S], f8, name="bpeg")
            nc.sync.dma_start(t[:], d_bpe[:, g * NGRAN:(g + 1) * NGRAN, :])
            bpe_tiles[g] = t
            return t

        # ---------------- phase P: projections ----------------
        with tc.tile_pool(name="w8", bufs=1) as w8_p, \
             tc.tile_pool(name="psP", bufs=3, space=bass.MemorySpace.PSUM) \
                as psP, \
             tc.tile_pool(name="psG", bufs=2, space=bass.MemorySpace.PSUM) \
                as psG:
            W8 = {}
            for w in ("Wq", "Wk", "Wv"):
                W8[w] = w8_p.tile([P, HC, H], f8, name="W8" + w)
                nc.sync.dma_start(W8[w][:], d_W8[w][:].transpose([1, 0, 2]))
            hidT8 = w8_p.tile([P, HC, S], f8)
            nc.sync.dma_start(hidT8[:], d_hidT8[:].transpose([1, 0, 2]))
            hidRT8 = w8_p.tile([P, HC, I_CORE], f8)
            nc.sync.dma_start(hidRT8[:], d_hidRT8[:].transpose([1, 0, 2]))

            fetch_gran(0)
            fetch_gran(1)

            # Q projection (transposed): psum = 16*(Wq @ hidR^T)
            for r in range(HC):
                pq = psP.tile([P, 512], f32, name="pp")
                for kc in range(HC):
                    nc.tensor.matmul(pq[:, 0:I_CORE],
                                     W8["Wq"][:, kc, r * P:(r + 1) * P],
                                     hidRT8[:, kc, :],
                                     start=(kc == 0), stop=(kc == HC - 1))
                for sub in range(2):
                    n = 2 * r + sub
                    src = pq[sub * DH:(sub + 1) * DH, 0:I_CORE]
                    bq_s = bqcol[sub * DH:(sub + 1) * DH, r:r + 1]
                    nc.scalar.activation(qT[:, n, :], src, Act.Identity,
                                         scale=1.0 / WSCALE, bias=bq_s)
                    bq4_s = bqcol4[sub * DH:(sub + 1) * DH, r:r + 1]
                    for par in range(2):
                        nc.scalar.activation(
                            qPair8[sub * DH:(sub + 1) * DH, :, 2 * n + par],
                            pq[sub * DH:(sub + 1) * DH, par:I_CORE:2],
                            Act.Identity, scale=4.0 / WSCALE, bias=bq4_s)

            # K projection (transposed): kT = Wk @ hid^T + bk
            for r in range(HC):
                for jh in range(2):
                    pk = psP.tile([P, 512], f32, name="pp")
                    for kc in range(HC):
                        nc.tensor.matmul(pk[:], W8["Wk"][:, kc, r * P:(r + 1) * P],
                                         hidT8[:, kc, jh * 512:(jh + 1) * 512],
                                         start=(kc == 0), stop=(kc == HC - 1))
                    for sub in range(2):
                        nc.scalar.activation(
                            kT[:, 2 * r + sub, jh * 512:(jh + 1) * 512],
                            pk[sub * DH:(sub + 1) * DH, :], Act.Identity,
                            scale=1.0 / WSCALE,
                            bias=bkcol[sub * DH:(sub + 1) * DH, r:r + 1])

            # V projection (natural): v = hid @ Wv^T + bv, + ones column
            for jc in range(SC):
                for vh in range(2):
                    pv = psP.tile([P, 512], f32, name="pp")
                    for kc in range(HC):
                        nc.tensor.matmul(pv[:, 0:VH],
                                         hidT8[:, kc, jc * P:(jc + 1) * P],
                                         W8["Wv"][:, kc, vh * VH:(vh + 1) * VH],
                                         start=(kc == 0), stop=False)
                    nc.tensor.matmul(pv[:, 0:VH], ones_row[:],
                                     bv16[:, vh * VH:(vh + 1) * VH],
                                     start=False, stop=True)
                    nc.vector.tensor_scalar(
                        v_sb[:, jc, vh * HP:(vh + 1) * HP, 0:DH],
                        pv[:, 0:VH].rearrange("p (a b) -> p a b", a=HP),
                        1.0 / WSCALE, None, Alu.mult)
            nc.vector.memset(v_sb[:, :, :, DH], 1.0)

            # gamma/beta broadcast via K=1 matmuls (own pool, end of phase)
            for c in range(HC):
                pbx = psG.tile([P, P], f32, name="pbx")
                nc.tensor.matmul(pbx[:], ones_row[:],
                                 gamma_r[:, c * P:(c + 1) * P])
                nc.scalar.copy(gammaB[:, c * P:(c + 1) * P], pbx[:])
                pbx2 = psG.tile([P, P], f32, name="pbx")
                nc.tensor.matmul(pbx2[:], ones_row[:],
                                 beta_r[:, c * P:(c + 1) * P])
                nc.scalar.copy(betaB[:, c * P:(c + 1) * P], pbx2[:])

        # ---------------- per-half phases ----------------
        for h in range(2):
            # ---- phase A: bias einsum + permuted transpose ----
            with tc.tile_pool(name="psB", bufs=2, space=bass.MemorySpace.PSUM) \
                    as psB, \
                 tc.tile_pool(name="psT", bufs=2, space=bass.MemorySpace.PSUM) \
                    as psT:
                biasT = biasT_p.tile([P, SC, NH, NOCT, 8], bf16, name="biasT")
                for g in range(8):
                    gg = h * 8 + g
                    if gg not in bpe_tiles:
                        fetch_gran(gg)
                    gt = bpe_tiles[gg]
                    if gg + 2 <= 15 and (gg + 2) not in bpe_tiles:
                        fetch_gran(gg + 2)
                    for o2 in range(2):
                        oct_ = g * 2 + o2
                        pb = psB.tile([P, 1024], f32, name="pb")
                        for c4 in range(4):
                            pr = h * (NPAIR // 2) + oct_ * 4 + c4
                            wi = pr % NGRAN
                            for jh in range(2):
                                nc.tensor.matmul(
                                    pb[32 * c4:32 * c4 + 32,
                                       jh * 512:(jh + 1) * 512],
                                    qPair8[:, pr, :],
                                    gt[:, wi, jh * 512:(jh + 1) * 512],
                                    tile_position=(0, 32 * c4))
                        b4t = b4_p.tile([P, S], bf16, name="b4")
                        nc.scalar.activation(b4t[:], pb[:], Act.Copy,
                                             scale=0.25)
                        ptb = psT.tile([P, SC, P], bf16, name="ptb")
                        for jc in range(SC):
                            nc.tensor.transpose(ptb[:, jc, :],
                                                b4t[:, jc * P:(jc + 1) * P],
                                                perm_bf[:])
                        nc.vector.tensor_copy(
                            biasT[:, :, :, oct_, :],
                            ptb[:, :, 0:96].rearrange(
                                "p a (b c) -> p a b c", b=NH))

            # ---- phase B: attention (QK + bias + softmax + PV fused) ----
            with tc.tile_pool(name="psS", bufs=2, space=bass.MemorySpace.PSUM) \
                    as psS, \
                 tc.tile_pool(name="psC", bufs=1, space=bass.MemorySpace.PSUM) \
                    as psC, \
                 tc.tile_pool(name="psR", bufs=1, space=bass.MemorySpace.PSUM) \
                    as psR:
                pctx = psC.tile([DH + 1, NH, IH], f32, name="pctx")
                for jc in range(SC):
                    for g2 in range(2):
                        n0 = g2 * HP
                        ps_s = psS.tile([P, HP, IH], f32, name="scores")
                        for nn in range(HP):
                            n = n0 + nn
                            nc.tensor.matmul(
                                ps_s[:, nn, :],
                                kT[:, n, jc * P:(jc + 1) * P],
                                qT[:, n, h * IH:(h + 1) * IH])
                        flat = ps_s[:].rearrange("p a b -> p (a b)")
                        nc.vector.tensor_tensor(
                            flat, flat,
                            biasT[:, jc, n0:n0 + HP].rearrange(
                                "p a b c -> p (a b c)"), Alu.add)
                        pt = probs_p.tile([P, HP, IH], bf16, name="probsT")
                        nc.scalar.activation(
                            pt[:].rearrange("p a b -> p (a b)"), flat,
                            Act.Exp, scale=0.125)
                        for nn in range(HP):
                            n = n0 + nn
                            nc.tensor.matmul(pctx[:, n, :],
                                             v_sb[:, jc, n, :],
                                             pt[:, nn, :],
                                             start=(jc == 0),
                                             stop=(jc == SC - 1),
                                             skip_group_check=True)

                sums = y_p.tile([1, NH, IH], f32, name="sums")
                nc.vector.tensor_copy(sums[0:1, :, :], pctx[DH:DH + 1, :, :])
                rec = y_p.tile([1, NH, IH], f32, name="rec")
                nc.vector.reciprocal(rec[:].rearrange("p a b -> p (a b)"),
                                     sums[:].rearrange("p a b -> p (a b)"))
                recB = y_p.tile([1, NH, IH], bf16, name="recB")
                nc.vector.tensor_copy(recB[:], rec[:])
                for n in range(NH):
                    hp, sub = n // 2, n % 2
                    pr = psR.tile([DH, IH], f32, name="prec")
                    nc.tensor.matmul(pr[:], ones_row[:, 0:DH], recB[0:1, n, :])
                    precS = y_p.tile([DH, IH], f32, name="precS")
                    nc.scalar.copy(precS[:], pr[:])
                    nc.vector.tensor_tensor(
                        ctxT[sub * DH:(sub + 1) * DH, hp, :],
                        pctx[0:DH, n, :], precS[:], Alu.mult)

                # ---- O proj + residual + LN ----
                y = y_p.tile([P, H], f32, name="yy")
                for vh in range(2):
                    # shares the "scores" slot rotation to stay in 8 banks
                    py = psS.tile([P, VH], f32, name="scores")
                    for kc in range(HC):
                        nc.tensor.matmul(py[:], ctxT[:, kc, :],
                                         WoT[:, kc, vh * VH:(vh + 1) * VH],
                                         start=(kc == 0), stop=False)
                    nc.tensor.matmul(py[:], ones_row[:],
                                     bo_bf[:, vh * VH:(vh + 1) * VH],
                                     start=False, stop=True)
                    nc.vector.tensor_tensor(y[:, vh * VH:(vh + 1) * VH],
                                            py[:],
                                            hidR[:, h, vh * VH:(vh + 1) * VH],
                                            Alu.add)
                mu = y_p.tile([P, 1], f32, name="mu")
                nc.vector.tensor_reduce(mu[:], y[:], AxisX, Alu.add)
                nc.vector.tensor_scalar(mu[:], mu[:], 1.0 / H, None, Alu.mult)
                yc = y_p.tile([P, H], f32, name="yc")
                nc.vector.tensor_scalar(yc[:], y[:], mu[:], None, Alu.subtract)
                ssq = y_p.tile([P, 1], f32, name="ssq")
                nc.scalar.activation(y[:], yc[:], Act.Square, accum_out=ssq[:])
                std = y_p.tile([P, 1], f32, name="std")
                nc.scalar.activation(std[:], ssq[:], Act.Sqrt,
                                     scale=1.0 / H, bias=eps_t[:])
                rstd = y_p.tile([P, 1], f32, name="rstd")
                nc.vector.reciprocal(rstd[:], std[:])
                o1 = y_p.tile([P, H], f32, name="o1")
                nc.vector.tensor_scalar(o1[:], yc[:], rstd[:], None, Alu.mult)
                nc.vector.tensor_tensor(o1[:], o1[:], gammaB[:], Alu.mult)
                nc.vector.tensor_tensor(o1[:], o1[:], betaB[:], Alu.add)
                nc.sync.dma_start(d_out[h], o1[:])

    nc.compile()
    return nc


def _build_perm():
    """Permutation: transpose output column f <- b4 strip row sigma(f).
    f-order: (n, c4, par) for f<96; sigma(f) = 32*c4 + 2*n + par."""
    perm = np.zeros((P, P), np.float32)
    for n in range(NH):
        for c4 in range(4):
            for par in range(2):
                f = n * 8 + c4 * 2 + par
                perm[32 * c4 + 2 * n + par, f] = 1.0
    for c4 in range(4):
        for u in range(8):
            perm[32 * c4 + 24 + u, 96 + c4 * 8 + u] = 1.0
    return perm


def _shard_inputs(inputs):
    import ml_dtypes
    bf = ml_dtypes.bfloat16
    f8 = ml_dtypes.float8_e3m4
    hs = np.ascontiguousarray(np.asarray(inputs["hidden_states"]), np.float32)
    bpe = np.asarray(inputs["bbox_pos_emb"])
    HC = H // P

    perm = _build_perm().astype(bf)
    W8 = {}
    for w in ("Wq", "Wk", "Wv"):
        W8[w] = np.ascontiguousarray(
            (np.asarray(inputs[w], np.float32).T * WSCALE).astype(f8)).reshape(
                HC, P, H)
    WoT = np.ascontiguousarray(
        np.asarray(inputs["Wo"], np.float32).T.astype(bf)).reshape(HC, P, H)

    def col(name, scale=1.0):
        v = np.asarray(inputs[name], np.float32) * scale
        return np.ascontiguousarray(v.reshape(HC, P).T)  # [P, HC]

    bqcol = col("bq")
    bqcol4 = col("bq", 4.0)
    bkcol = col("bk")
    bv16 = (np.asarray(inputs["bv"], np.float32) * WSCALE).astype(bf).reshape(1, H)
    bo_bf = np.asarray(inputs["bo"], np.float32).astype(bf).reshape(1, H)
    gamma_bf = np.asarray(inputs["ln_gamma"], np.float32).astype(bf).reshape(1, H)
    beta_bf = np.asarray(inputs["ln_beta"], np.float32).astype(bf).reshape(1, H)

    hsT8 = {b: np.ascontiguousarray(hs[b].T).astype(f8).reshape(HC, P, S)
            for b in range(B)}

    in_maps = []
    for c in range(N_CORES):
        b = c // 4
        q0 = (c % 4) * I_CORE
        # bpe8 [128, 128, 1024]: row p=(i%2)*64+d, [pair, j]
        x = bpe[q0:q0 + I_CORE, :, b, :]            # [256 i, 1024 j, 64 d]
        x = np.asarray(x, np.float32).reshape(I_CORE // 2, 2, S, DH)
        x = x.transpose(1, 3, 0, 2).reshape(P, I_CORE // 2, S)
        m = {
            "hidT8": hsT8[b],
            "hidRT8": np.ascontiguousarray(
                hs[b, q0:q0 + I_CORE].T).astype(f8).reshape(HC, P, I_CORE),
            "hidR": np.ascontiguousarray(
                hs[b, q0:q0 + I_CORE].reshape(2, P, H)),
            "bpe8": np.ascontiguousarray(x).astype(f8),
            "WoT": WoT,
            "bqcol": bqcol, "bqcol4": bqcol4, "bkcol": bkcol,
            "bv16": bv16, "bo_bf": bo_bf,
            "gamma_bf": gamma_bf, "beta_bf": beta_bf,
            "perm_bf": perm,
        }
        for w in ("Wq", "Wk", "Wv"):
            m[w + "8"] = W8[w]
        in_maps.append(m)
    return in_maps


def _install_ntff_shim():
    """The agent image's antenv lacks axon_hooks; recreate the NTFF profile
    hook via ctypes against libaxon_pjrt.so so trace=True yields
    exec_time_ns + a perfetto trace."""
    import sys as _sys
    if "antenv.axon_hooks" in _sys.modules:
        return
    import types, ctypes, contextlib
    so_path = "/opt/axon/libaxon_pjrt.so"
    mod = types.ModuleType("antenv.axon_hooks")
    _state = {}

    def get_axon_ntff_profile_hook():
        if "hook" in _state:
            return _state["hook"]
        try:
            lib = ctypes.CDLL(so_path)
            if not hasattr(lib, "axon_start_nrt_profile"):
                _state["hook"] = None
                return None
            lib.axon_start_nrt_profile.argtypes = [
                ctypes.POINTER(ctypes.c_int64), ctypes.c_size_t]
            lib.axon_start_nrt_profile.restype = ctypes.c_int64
            lib.axon_stop_nrt_profile.argtypes = [ctypes.c_char_p]
            lib.axon_stop_nrt_profile.restype = ctypes.c_int64
        except OSError:
            _state["hook"] = None
            return None

        @contextlib.contextmanager
        def _hook(output_dir, device_ids):
            import jax
            jax.devices()
            if device_ids:
                ids = (ctypes.c_int64 * len(device_ids))(*device_ids)
                rc = lib.axon_start_nrt_profile(ids, len(device_ids))
            else:
                rc = lib.axon_start_nrt_profile(None, 0)
            if rc != 0:
                raise RuntimeError(f"axon_start_nrt_profile rc={rc}")
            try:
                yield
            finally:
                n = lib.axon_stop_nrt_profile(str(output_dir).encode())
                print(f"ntff profile: {n} file(s) written to {output_dir}")

        _state["hook"] = _hook
        return _hook

    mod.get_axon_ntff_profile_hook = get_axon_ntff_profile_hook
    _sys.modules["antenv.axon_hooks"] = mod


def kernel(**inputs):
    from concourse.bass_utils import run_bass_kernel_spmd

    if os.environ.get("BASS_KERNEL_TRACE"):
        _install_ntff_shim()
        import concourse.bass_utils as _bu
        _bu.upload_artifacts = lambda tmpdir: f"file://{tmpdir}"

    if "nc" not in _COMPILED:
        _COMPILED["nc"] = build_kernel()
    nc = _COMPILED["nc"]
    in_maps = _shard_inputs(inputs)
    res = run_bass_kernel_spmd(nc, in_maps, core_ids=list(range(N_CORES)),
                               trace=bool(os.environ.get("BASS_KERNEL_TRACE")))
    _COMPILED["last_result"] = res
    out = np.zeros((B, S, H), dtype=np.float32)
    for c in range(N_CORES):
        b = c // 4
        q0 = (c % 4) * I_CORE
        out[b, q0:q0 + I_CORE] = np.asarray(
            res.results[c]["out"]).reshape(I_CORE, H)
    return out


# revision 26
# speedup vs baseline: 1.7822x; 1.1275x over previous
"""Distributed Trainium2 Bass kernel for BrosAttention.

B=2, S=1024, H=768, NH=12, DH=64:
  q,k,v = heads(hidden @ W.T + b)
  scores = q@k^T + einsum('bnid,bijd->bnij', q, bpe)   (bpe = bbox transposed)
  probs  = softmax(scores / 8)
  out    = LN(probs@v @ Wo.T + bo + hidden)

Sharding: 8 cores = 2 batches x 4 query-row blocks of 256 rows; each core
reads only its slice of bbox_pos_emb (fp8 e3m4, 16.8MB) and writes a
disjoint [256, 768] output block. No collectives.

Structure:
 - fp8 (e3m4) inputs for projections + bias einsum; weights pre-scaled x16
   on host, descaled inside the PSUM-copy activations.
 - Bias einsum: block-diag qPair weights [128,32] (2 query rows x 12 heads,
   n-major columns), 4 pairs concurrent in PE column strips; strips
   transposed back through a host-built permutation matrix so the result
   comes out n-grouped -> the scores+bias add is one contiguous
   tensor_tensor (in-place in PSUM) per [j-chunk, 6-head group].
 - kT/qT stored as 64-partition tiles: every QK matmul reads partition
   base 0 (base-64 operands + offset PSUM writes crash the HW).
 - Softmax sums folded into P@V as a 65th all-ones column of V; PV is
   interleaved with QK per j-chunk, accumulating into a persistent
   [65, 12, 128] psum tile.
"""

import os
import sys
import numpy as np

sys.path.insert(0, "/opt/trn_rl_repo")

B, S, H, NH, DH = 2, 1024, 768, 12, 64
EPS = 1e-12
P = 128
I_CORE = S * B // 8  # 256
N_CORES = 8
WSCALE = 16.0

_COMPILED = {}


def build_kernel():
    from contextlib import ExitStack
    from concourse import bacc, bass, mybir, tile

    f32 = mybir.dt.float32
    bf16 = mybir.dt.bfloat16
    f8 = mybir.dt.float8e3
    Alu = mybir.AluOpType
    Act = mybir.ActivationFunctionType
    AxisX = mybir.AxisListType.X

    SC = S // P            # 8 j chunks
    HC = H // P            # 6 hidden chunks
    IH = I_CORE // 2       # 128 i per half
    NPAIR = I_CORE // 2    # 128 i-pairs per core
    NOCT = 16              # octos (8 i's) per half
    NGRAN = 8              # bpe granule = 8 pairs (1.05 MB DMA)
    HP = NH // 2
    VH = H // 2            # 384

    nc = bacc.Bacc(None, target_bir_lowering=False, debug=False)

    d_hidT = nc.declare_dram_parameter("hidT_bf", [HC, P, S], bf16, isOutput=False)
    d_hidRT8 = nc.declare_dram_parameter("hidRT8", [HC, P, I_CORE], f8, isOutput=False)
    d_hidR = nc.declare_dram_parameter("hidR", [2, P, H], f32, isOutput=False)
    d_bpe = nc.declare_dram_parameter("bpe8", [P, NPAIR, S], f8, isOutput=False)
    d_Wq8 = nc.declare_dram_parameter("Wq8", [HC, P, H], f8, isOutput=False)
    d_Wk = nc.declare_dram_parameter("WkT_bf", [HC, P, H], bf16, isOutput=False)
    d_Wv = nc.declare_dram_parameter("WvT_bf", [HC, P, H], bf16, isOutput=False)
    d_WoT = nc.declare_dram_parameter("WoT", [HC, P, H], bf16, isOutput=False)
    d_bqcol = nc.declare_dram_parameter("bqcol", [P, HC], f32, isOutput=False)
    d_bqcol4 = nc.declare_dram_parameter("bqcol4", [P, HC], f32, isOutput=False)
    d_bkcol = nc.declare_dram_parameter("bkcol", [P, HC], f32, isOutput=False)
    d_bv = nc.declare_dram_parameter("bv_bf", [1, H], bf16, isOutput=False)
    d_bo = nc.declare_dram_parameter("bo_bf", [1, H], bf16, isOutput=False)
    d_gamma = nc.declare_dram_parameter("gamma_bf", [1, H], bf16, isOutput=False)
    d_beta = nc.declare_dram_parameter("beta_bf", [1, H], bf16, isOutput=False)
    d_perm = nc.declare_dram_parameter("perm_bf", [P, P], bf16, isOutput=False)
    d_out = nc.declare_dram_parameter("out", [2, P, H], f32, isOutput=True)

    with tile.TileContext(nc) as tc, ExitStack() as ctx:
        const_p = ctx.enter_context(tc.tile_pool(name="const", bufs=1))
        stat_p = ctx.enter_context(tc.tile_pool(name="stat", bufs=1))
        bpe_p = ctx.enter_context(tc.tile_pool(name="bpe", bufs=3))
        biasT_p = ctx.enter_context(tc.tile_pool(name="biasT", bufs=1))
        b4_p = ctx.enter_context(tc.tile_pool(name="b4", bufs=2))
        probs_p = ctx.enter_context(tc.tile_pool(name="probs", bufs=3))
        y_p = ctx.enter_context(tc.tile_pool(name="y", bufs=1))

        # ---------------- constants ----------------
        perm_bf = const_p.tile([P, P], bf16)
        nc.sync.dma_start(perm_bf[:], d_perm[:])
        ones_row = const_p.tile([1, P], bf16)
        nc.vector.memset(ones_row[:], 1.0)
        eps_t = const_p.tile([P, 1], f32)
        nc.vector.memset(eps_t[:], EPS)
        bqcol = const_p.tile([P, HC], f32)
        nc.sync.dma_start(bqcol[:], d_bqcol[:])
        bqcol4 = const_p.tile([P, HC], f32)
        nc.sync.dma_start(bqcol4[:], d_bqcol4[:])
        bkcol = const_p.tile([P, HC], f32)
        nc.sync.dma_start(bkcol[:], d_bkcol[:])
        bv_bf = const_p.tile([1, H], bf16)
        nc.sync.dma_start(bv_bf[:], d_bv[:])
        bo_bf = const_p.tile([1, H], bf16)
        nc.sync.dma_start(bo_bf[:], d_bo[:])
        gamma_r = const_p.tile([1, H], bf16)
        nc.sync.dma_start(gamma_r[:], d_gamma[:])
        beta_r = const_p.tile([1, H], bf16)
        nc.sync.dma_start(beta_r[:], d_beta[:])

        # long-lived activations (kT/qT: 64-partition tiles, base-0 reads)
        kT = stat_p.tile([DH, NH, S], bf16)
        v_sb = stat_p.tile([P, SC, NH, DH + 1], bf16)
        qT = stat_p.tile([DH, NH, I_CORE], bf16)
        qPair8 = stat_p.tile([P, NPAIR, 32], f8)    # block-diag bias weights
        nc.vector.memset(qPair8[:], 0.0)
        hidR = stat_p.tile([P, 2, H], f32)
        nc.sync.dma_start(hidR[:], d_hidR[:].transpose([1, 0, 2]))
        WoT = stat_p.tile([P, HC, H], bf16)
        nc.sync.dma_start(WoT[:], d_WoT[:].transpose([1, 0, 2]))
        gammaB = stat_p.tile([P, H], bf16)
        betaB = stat_p.tile([P, H], bf16)
        ctxT = stat_p.tile([P, HC, IH], bf16)

        # bpe granule streaming
        bpe_tiles = {}

        def fetch_gran(g):
            t = bpe_p.tile([P, NGRAN, S], f8, name="bpeg")
            nc.sync.dma_start(t[:], d_bpe[:, g * NGRAN:(g + 1) * NGRAN, :])
            bpe_tiles[g] = t
            return t

        # ---------------- phase P: projections ----------------
        with tc.tile_pool(name="w8", bufs=1) as w8_p, \
             tc.tile_pool(name="psP", bufs=3, space=bass.MemorySpace.PSUM) \
                as psP, \
             tc.tile_pool(name="psG", bufs=2, space=bass.MemorySpace.PSUM) \
                as psG:
            Wq8 = w8_p.tile([P, HC, H], f8)
            nc.sync.dma_start(Wq8[:], d_Wq8[:].transpose([1, 0, 2]))
            hidRT8 = w8_p.tile([P, HC, I_CORE], f8)
            nc.sync.dma_start(hidRT8[:], d_hidRT8[:].transpose([1, 0, 2]))
            WkT = w8_p.tile([P, HC, H], bf16)
            nc.sync.dma_start(WkT[:], d_Wk[:].transpose([1, 0, 2]))
            WvT = w8_p.tile([P, HC, H], bf16)
            nc.sync.dma_start(WvT[:], d_Wv[:].transpose([1, 0, 2]))
            hidT = w8_p.tile([P, HC, S], bf16)
            nc.sync.dma_start(hidT[:], d_hidT[:].transpose([1, 0, 2]))

            fetch_gran(0)
            fetch_gran(1)

            # Q projection (transposed): psum = 16*(Wq @ hidR^T)
            for r in range(HC):
                pq = psP.tile([P, 512], f32, name="pp")
                for kc in range(HC):
                    nc.tensor.matmul(pq[:, 0:I_CORE],
                                     Wq8[:, kc, r * P:(r + 1) * P],
                                     hidRT8[:, kc, :],
                                     start=(kc == 0), stop=(kc == HC - 1))
                for sub in range(2):
                    n = 2 * r + sub
                    srcp = pq[sub * DH:(sub + 1) * DH, 0:I_CORE]
                    bq_s = bqcol[sub * DH:(sub + 1) * DH, r:r + 1]
                    nc.vector.tensor_scalar(qT[:, n, :], srcp, 1.0 / WSCALE,
                                            bq_s, Alu.mult, Alu.add)
                    bq4_s = bqcol4[sub * DH:(sub + 1) * DH, r:r + 1]
                    nc.scalar.activation(
                        qPair8[sub * DH:(sub + 1) * DH, :, 2 * n:2 * n + 2],
                        srcp.rearrange("p (a b) -> p a b", b=2),
                        Act.Identity, scale=4.0 / WSCALE, bias=bq4_s)

            # K projection (transposed): kT = Wk @ hid^T + bk
            for r in range(HC):
                for jh in range(2):
                    pk = psP.tile([P, 512], f32, name="pp")
                    for kc in range(HC):
                        nc.tensor.matmul(pk[:], WkT[:, kc, r * P:(r + 1) * P],
                                         hidT[:, kc, jh * 512:(jh + 1) * 512],
                                         start=(kc == 0), stop=(kc == HC - 1))
                    for sub in range(2):
                        nc.vector.tensor_scalar(
                            kT[:, 2 * r + sub, jh * 512:(jh + 1) * 512],
                            pk[sub * DH:(sub + 1) * DH, :],
                            bkcol[sub * DH:(sub + 1) * DH, r:r + 1], None,
                            Alu.add)

            # V projection (natural): v = hid @ Wv^T + bv, + ones column
            for jc in range(SC):
                for vh in range(2):
                    pv = psP.tile([P, 512], f32, name="pp")
                    for kc in range(HC):
                        nc.tensor.matmul(pv[:, 0:VH],
                                         hidT[:, kc, jc * P:(jc + 1) * P],
                                         WvT[:, kc, vh * VH:(vh + 1) * VH],
                                         start=(kc == 0), stop=False)
                    nc.tensor.matmul(pv[:, 0:VH], ones_row[:],
                                     bv_bf[:, vh * VH:(vh + 1) * VH],
                                     start=False, stop=True)
                    nc.vector.tensor_copy(
                        v_sb[:, jc, vh * HP:(vh + 1) * HP, 0:DH],
                        pv[:, 0:VH].rearrange("p (a b) -> p a b", a=HP))
            nc.vector.memset(v_sb[:, :, :, DH], 1.0)

            # gamma/beta broadcast via K=1 matmuls (own pool, end of phase)
            for c in range(HC):
                pbx = psG.tile([P, P], f32, name="pbx")
                nc.tensor.matmul(pbx[:], ones_row[:],
                                 gamma_r[:, c * P:(c + 1) * P])
                nc.scalar.copy(gammaB[:, c * P:(c + 1) * P], pbx[:])
                pbx2 = psG.tile([P, P], f32, name="pbx")
                nc.tensor.matmul(pbx2[:], ones_row[:],
                                 beta_r[:, c * P:(c + 1) * P])
                nc.scalar.copy(betaB[:, c * P:(c + 1) * P], pbx2[:])

        # ---------------- per-half phases ----------------
        for h in range(2):
            # ---- phase A: bias einsum + permuted transpose ----
            with tc.tile_pool(name="psB", bufs=2, space=bass.MemorySpace.PSUM) \
                    as psB, \
                 tc.tile_pool(name="psT", bufs=2, space=bass.MemorySpace.PSUM) \
                    as psT:
                biasT = biasT_p.tile([P, SC, NH, NOCT, 8], bf16, name="biasT")
                for g in range(8):
                    gg = h * 8 + g
                    if gg not in bpe_tiles:
                        fetch_gran(gg)
                    gt = bpe_tiles[gg]
                    if gg + 2 <= 15 and (gg + 2) not in bpe_tiles:
                        fetch_gran(gg + 2)
                    for o2 in range(2):
                        oct_ = g * 2 + o2
                        pb = psB.tile([P, 1024], f32, name="pb")
                        for c4 in range(4):
                            pr = h * (NPAIR // 2) + oct_ * 4 + c4
                            wi = pr % NGRAN
                            for jh in range(2):
                                nc.tensor.matmul(
                                    pb[32 * c4:32 * c4 + 32,
                                       jh * 512:(jh + 1) * 512],
                                    qPair8[:, pr, :],
                                    gt[:, wi, jh * 512:(jh + 1) * 512],
                                    tile_position=(0, 32 * c4))
                        b4t = b4_p.tile([P, S], bf16, name="b4")
                        nc.scalar.activation(b4t[:], pb[:], Act.Copy,
                                             scale=0.25)
                        ptb = psT.tile([P, SC, P], bf16, name="ptb")
                        for jc in range(SC):
                            nc.tensor.transpose(ptb[:, jc, :],
                                                b4t[:, jc * P:(jc + 1) * P],
                                                perm_bf[:])
                        nc.vector.tensor_copy(
                            biasT[:, :, :, oct_, :],
                            ptb[:, :, 0:96].rearrange(
                                "p a (b c) -> p a b c", b=NH))

            # ---- phase B: attention (QK + bias + softmax + PV fused) ----
            with tc.tile_pool(name="psS", bufs=2, space=bass.MemorySpace.PSUM) \
                    as psS, \
                 tc.tile_pool(name="psC", bufs=1, space=bass.MemorySpace.PSUM) \
                    as psC:
                pctx = psC.tile([DH + 1, NH, IH], f32, name="pctx")
                for jc in range(SC):
                    for g2 in range(2):
                        n0 = g2 * HP
                        ps_s = psS.tile([P, HP, IH], f32, name="scores")
                        for nn in range(HP):
                            n = n0 + nn
                            nc.tensor.matmul(
                                ps_s[:, nn, :],
                                kT[:, n, jc * P:(jc + 1) * P],
                                qT[:, n, h * IH:(h + 1) * IH])
                        flat = ps_s[:].rearrange("p a b -> p (a b)")
                        nc.vector.tensor_tensor(
                            flat, flat,
                            biasT[:, jc, n0:n0 + HP].rearrange(
                                "p a b c -> p (a b c)"), Alu.add)
                        pt = probs_p.tile([P, HP, IH], bf16, name="probsT")
                        nc.scalar.activation(
                            pt[:].rearrange("p a b -> p (a b)"), flat,
                            Act.Exp, scale=0.125)
                        for nn in range(HP):
                            n = n0 + nn
                            nc.tensor.matmul(pctx[:, n, :],
                                             v_sb[:, jc, n, :],
                                             pt[:, nn, :],
                                             start=(jc == 0),
                                             stop=(jc == SC - 1),
                                             skip_group_check=True)

                sumsB = y_p.tile([1, NH, IH], bf16, name="sumsB")
                nc.vector.tensor_copy(sumsB[0:1, :, :], pctx[DH:DH + 1, :, :])
                for g2 in range(2):
                    prs = psS.tile([DH, HP, IH], f32, name="scores")
                    for nn in range(HP):
                        nc.tensor.matmul(prs[:, nn, :], ones_row[:, 0:DH],
                                         sumsB[0:1, g2 * HP + nn, :])
                    precS = y_p.tile([DH, HP, IH], f32, name="precS")
                    nc.vector.reciprocal(
                        precS[:].rearrange("p a b -> p (a b)"),
                        prs[:].rearrange("p a b -> p (a b)"))
                    for sub in range(2):
                        nc.vector.tensor_tensor(
                            ctxT[sub * DH:(sub + 1) * DH,
                                 3 * g2:3 * g2 + 3, :],
                            pctx[0:DH, g2 * HP + sub:g2 * HP + HP:2, :],
                            precS[:, sub:HP:2, :], Alu.mult)

                # ---- O proj + residual + LN ----
                y = y_p.tile([P, H], f32, name="yy")
                for vh in range(2):
                    # shares the "scores" slot rotation to stay in 8 banks
                    py = psS.tile([P, VH], f32, name="scores")
                    for kc in range(HC):
                        nc.tensor.matmul(py[:], ctxT[:, kc, :],
                                         WoT[:, kc, vh * VH:(vh + 1) * VH],
                                         start=(kc == 0), stop=False)
                    nc.tensor.matmul(py[:], ones_row[:],
                                     bo_bf[:, vh * VH:(vh + 1) * VH],
                                     start=False, stop=True)
                    nc.vector.tensor_tensor(y[:, vh * VH:(vh + 1) * VH],
                                            py[:],
                                            hidR[:, h, vh * VH:(vh + 1) * VH],
                                            Alu.add)
                mu = y_p.tile([P, 1], f32, name="mu")
                nc.vector.tensor_reduce(mu[:], y[:], AxisX, Alu.add)
                nc.vector.tensor_scalar(mu[:], mu[:], 1.0 / H, None, Alu.mult)
                yc = y_p.tile([P, H], f32, name="yc")
                nc.vector.tensor_scalar(yc[:], y[:], mu[:], None, Alu.subtract)
                ssq = y_p.tile([P, 1], f32, name="ssq")
                nc.scalar.activation(y[:], yc[:], Act.Square, accum_out=ssq[:])
                std = y_p.tile([P, 1], f32, name="std")
                nc.scalar.activation(std[:], ssq[:], Act.Sqrt,
                                     scale=1.0 / H, bias=eps_t[:])
                rstd = y_p.tile([P, 1], f32, name="rstd")
                nc.vector.reciprocal(rstd[:], std[:])
                o1 = y_p.tile([P, H], f32, name="o1")
                nc.vector.tensor_scalar(o1[:], yc[:], rstd[:], None, Alu.mult)
                nc.vector.tensor_tensor(o1[:], o1[:], gammaB[:], Alu.mult)
                nc.vector.tensor_tensor(o1[:], o1[:], betaB[:], Alu.add)
                nc.sync.dma_start(d_out[h], o1[:])

    nc.compile()
    return nc


def _build_perm():
    """Permutation: transpose output column f <- b4 strip row sigma(f).
    f-order: (n, c4, par) for f<96; sigma(f) = 32*c4 + 2*n + par."""
    perm = np.zeros((P, P), np.float32)
    for n in range(NH):
        for c4 in range(4):
            for par in range(2):
                f = n * 8 + c4 * 2 + par
                perm[32 * c4 + 2 * n + par, f] = 1.0
    for c4 in range(4):
        for u in range(8):
            perm[32 * c4 + 24 + u, 96 + c4 * 8 + u] = 1.0
    return perm


def _shard_inputs(inputs):
    import ml_dtypes
    bf = ml_dtypes.bfloat16
    f8 = ml_dtypes.float8_e3m4
    hs = np.ascontiguousarray(np.asarray(inputs["hidden_states"]), np.float32)
    bpe = np.asarray(inputs["bbox_pos_emb"])
    HC = H // P

    perm = _build_perm().astype(bf)
    Wq8 = np.ascontiguousarray(
        (np.asarray(inputs["Wq"], np.float32).T * WSCALE).astype(f8)).reshape(
            HC, P, H)
    WkT = np.ascontiguousarray(
        np.asarray(inputs["Wk"], np.float32).T.astype(bf)).reshape(HC, P, H)
    WvT = np.ascontiguousarray(
        np.asarray(inputs["Wv"], np.float32).T.astype(bf)).reshape(HC, P, H)
    WoT = np.ascontiguousarray(
        np.asarray(inputs["Wo"], np.float32).T.astype(bf)).reshape(HC, P, H)

    def col(name, scale=1.0):
        v = np.asarray(inputs[name], np.float32) * scale
        return np.ascontiguousarray(v.reshape(HC, P).T)  # [P, HC]

    bqcol = col("bq")
    bqcol4 = col("bq", 4.0)
    bkcol = col("bk")
    bv_bf = np.asarray(inputs["bv"], np.float32).astype(bf).reshape(1, H)
    bo_bf = np.asarray(inputs["bo"], np.float32).astype(bf).reshape(1, H)
    gamma_bf = np.asarray(inputs["ln_gamma"], np.float32).astype(bf).reshape(1, H)
    beta_bf = np.asarray(inputs["ln_beta"], np.float32).astype(bf).reshape(1, H)

    hsT = {b: np.ascontiguousarray(hs[b].T).astype(bf).reshape(HC, P, S)
           for b in range(B)}

    in_maps = []
    for c in range(N_CORES):
        b = c // 4
        q0 = (c % 4) * I_CORE
        # bpe8 [128, 128, 1024]: row p=(i%2)*64+d, [pair, j]
        x = bpe[q0:q0 + I_CORE, :, b, :]            # [256 i, 1024 j, 64 d]
        x = np.asarray(x, np.float32).reshape(I_CORE // 2, 2, S, DH)
        x = x.transpose(1, 3, 0, 2).reshape(P, I_CORE // 2, S)
        m = {
            "hidT_bf": hsT[b],
            "hidRT8": np.ascontiguousarray(
                hs[b, q0:q0 + I_CORE].T).astype(f8).reshape(HC, P, I_CORE),
            "hidR": np.ascontiguousarray(
                hs[b, q0:q0 + I_CORE].reshape(2, P, H)),
            "bpe8": np.ascontiguousarray(x).astype(f8),
            "WoT": WoT, "Wq8": Wq8, "WkT_bf": WkT, "WvT_bf": WvT,
            "bqcol": bqcol, "bqcol4": bqcol4, "bkcol": bkcol,
            "bv_bf": bv_bf, "bo_bf": bo_bf,
            "gamma_bf": gamma_bf, "beta_bf": beta_bf,
            "perm_bf": perm,
        }
        in_maps.append(m)
    return in_maps


def _install_ntff_shim():
    """The agent image's antenv lacks axon_hooks; recreate the NTFF profile
    hook via ctypes against libaxon_pjrt.so so trace=True yields
    exec_time_ns + a perfetto trace."""
    import sys as _sys
    if "antenv.axon_hooks" in _sys.modules:
        return
    import types, ctypes, contextlib
    so_path = "/opt/axon/libaxon_pjrt.so"
    mod = types.ModuleType("antenv.axon_hooks")
    _state = {}

    def get_axon_ntff_profile_hook():
        if "hook" in _state:
            return _state["hook"]
        try:
            lib = ctypes.CDLL(so_path)
            if not hasattr(lib, "axon_start_nrt_profile"):
                _state["hook"] = None
                return None
            lib.axon_start_nrt_profile.argtypes = [
                ctypes.POINTER(ctypes.c_int64), ctypes.c_size_t]
            lib.axon_start_nrt_profile.restype = ctypes.c_int64
            lib.axon_stop_nrt_profile.argtypes = [ctypes.c_char_p]
            lib.axon_stop_nrt_profile.restype = ctypes.c_int64
        except OSError:
            _state["hook"] = None
            return None

        @contextlib.contextmanager
        def _hook(output_dir, device_ids):
            import jax
            jax.devices()
            if device_ids:
                ids = (ctypes.c_int64 * len(device_ids))(*device_ids)
                rc = lib.axon_start_nrt_profile(ids, len(device_ids))
            else:
                rc = lib.axon_start_nrt_profile(None, 0)
            if rc != 0:
                raise RuntimeError(f"axon_start_nrt_profile rc={rc}")
            try:
                yield
            finally:
                n = lib.axon_stop_nrt_profile(str(output_dir).encode())
                print(f"ntff profile: {n} file(s) written to {output_dir}")

        _state["hook"] = _hook
        return _hook

    mod.get_axon_ntff_profile_hook = get_axon_ntff_profile_hook
    _sys.modules["antenv.axon_hooks"] = mod


def kernel(**inputs):
    from concourse.bass_utils import run_bass_kernel_spmd

    if os.environ.get("BASS_KERNEL_TRACE"):
        _install_ntff_shim()
        import concourse.bass_utils as _bu
        _bu.upload_artifacts = lambda tmpdir: f"file://{tmpdir}"

    if "nc" not in _COMPILED:
        _COMPILED["nc"] = build_kernel()
    nc = _COMPILED["nc"]
    in_maps = _shard_inputs(inputs)
    res = run_bass_kernel_spmd(nc, in_maps, core_ids=list(range(N_CORES)),
                               trace=bool(os.environ.get("BASS_KERNEL_TRACE")))
    _COMPILED["last_result"] = res
    out = np.zeros((B, S, H), dtype=np.float32)
    for c in range(N_CORES):
        b = c // 4
        q0 = (c % 4) * I_CORE
        out[b, q0:q0 + I_CORE] = np.asarray(
            res.results[c]["out"]).reshape(I_CORE, H)
    return out


# revision 27
# speedup vs baseline: 2.0171x; 1.1318x over previous
"""Distributed Trainium2 Bass kernel for BrosAttention.

B=2, S=1024, H=768, NH=12, DH=64:
  q,k,v = heads(hidden @ W.T + b)
  scores = q@k^T + einsum('bnid,bijd->bnij', q, bpe)   (bpe = bbox transposed)
  probs  = softmax(scores / 8)
  out    = LN(probs@v @ Wo.T + bo + hidden)

Sharding: 8 cores = 2 batches x 4 query-row blocks of 256 rows; each core
reads only its slice of bbox_pos_emb (fp8 e3m4, 16.8MB) and writes a
disjoint [256, 768] output block. No collectives.

Structure:
 - fp8 (e3m4) inputs for projections + bias einsum; weights pre-scaled x16
   on host, descaled inside the PSUM-copy activations.
 - Bias einsum: block-diag qPair weights [128,32] (2 query rows x 12 heads,
   n-major columns), 4 pairs concurrent in PE column strips; strips
   transposed back through a host-built permutation matrix so the result
   comes out n-grouped -> the scores+bias add is one contiguous
   tensor_tensor (in-place in PSUM) per [j-chunk, 6-head group].
 - kT/qT stored as 64-partition tiles: every QK matmul reads partition
   base 0 (base-64 operands + offset PSUM writes crash the HW).
 - Softmax sums folded into P@V as a 65th all-ones column of V; PV is
   interleaved with QK per j-chunk, accumulating into a persistent
   [65, 12, 128] psum tile.
"""

import os
import sys
import numpy as np

sys.path.insert(0, "/opt/trn_rl_repo")

B, S, H, NH, DH = 2, 1024, 768, 12, 64
EPS = 1e-12
P = 128
I_CORE = S * B // 8  # 256
N_CORES = 8
WSCALE = 16.0

_COMPILED = {}


def build_kernel():
    from contextlib import ExitStack
    from concourse import bacc, bass, mybir, tile

    f32 = mybir.dt.float32
    bf16 = mybir.dt.bfloat16
    f8 = mybir.dt.float8e3
    Alu = mybir.AluOpType
    Act = mybir.ActivationFunctionType
    AxisX = mybir.AxisListType.X

    SC = S // P            # 8 j chunks
    HC = H // P            # 6 hidden chunks
    IH = I_CORE // 2       # 128 i per half
    NPAIR = I_CORE // 2    # 128 i-pairs per core
    NOCT = 16              # octos (8 i's) per half
    NGRAN = 8              # bpe granule = 8 pairs (1.05 MB DMA)
    HP = NH // 2
    VH = H // 2            # 384

    nc = bacc.Bacc(None, target_bir_lowering=False, debug=False)

    d_hidT = nc.declare_dram_parameter("hidT_bf", [HC, P, S], bf16, isOutput=False)
    d_hidRT8 = nc.declare_dram_parameter("hidRT8", [HC, P, I_CORE], f8, isOutput=False)
    d_hidR = nc.declare_dram_parameter("hidR", [2, P, H], f32, isOutput=False)
    d_bpe = nc.declare_dram_parameter("bpe8", [P, NPAIR, S], f8, isOutput=False)
    d_Wq8 = nc.declare_dram_parameter("Wq8", [HC, P, H], f8, isOutput=False)
    d_Wk = nc.declare_dram_parameter("WkT_bf", [HC, P, H], bf16, isOutput=False)
    d_Wv = nc.declare_dram_parameter("WvT_bf", [HC, P, H], bf16, isOutput=False)
    d_WoT = nc.declare_dram_parameter("WoT", [HC, P, H], bf16, isOutput=False)
    d_bqcol = nc.declare_dram_parameter("bqcol", [P, HC], f32, isOutput=False)
    d_bqcol4 = nc.declare_dram_parameter("bqcol4", [P, HC], f32, isOutput=False)
    d_bkcol = nc.declare_dram_parameter("bkcol", [P, HC], f32, isOutput=False)
    d_bv = nc.declare_dram_parameter("bv_bf", [1, H], bf16, isOutput=False)
    d_bo = nc.declare_dram_parameter("bo_bf", [1, H], bf16, isOutput=False)
    d_gamma = nc.declare_dram_parameter("gamma_bf", [1, H], bf16, isOutput=False)
    d_beta = nc.declare_dram_parameter("beta_bf", [1, H], bf16, isOutput=False)
    d_perm = nc.declare_dram_parameter("perm_bf", [P, P], bf16, isOutput=False)
    d_out = nc.declare_dram_parameter("out", [2, P, H], f32, isOutput=True)

    with tile.TileContext(nc) as tc, ExitStack() as ctx:
        const_p = ctx.enter_context(tc.tile_pool(name="const", bufs=1))
        stat_p = ctx.enter_context(tc.tile_pool(name="stat", bufs=1))
        bpe_p = ctx.enter_context(tc.tile_pool(name="bpe", bufs=3))
        biasT_p = ctx.enter_context(tc.tile_pool(name="biasT", bufs=1))
        b4_p = ctx.enter_context(tc.tile_pool(name="b4", bufs=2))
        probs_p = ctx.enter_context(tc.tile_pool(name="probs", bufs=3))
        y_p = ctx.enter_context(tc.tile_pool(name="y", bufs=1))

        # ---------------- constants ----------------
        perm_bf = const_p.tile([P, P], bf16)
        nc.sync.dma_start(perm_bf[:], d_perm[:])
        ones_row = const_p.tile([1, P], bf16)
        nc.vector.memset(ones_row[:], 1.0)
        eps_t = const_p.tile([P, 1], f32)
        nc.vector.memset(eps_t[:], EPS)
        bqcol = const_p.tile([P, HC], f32)
        nc.sync.dma_start(bqcol[:], d_bqcol[:])
        bqcol4 = const_p.tile([P, HC], f32)
        nc.sync.dma_start(bqcol4[:], d_bqcol4[:])
        bkcol = const_p.tile([P, HC], f32)
        nc.sync.dma_start(bkcol[:], d_bkcol[:])
        bv_bf = const_p.tile([1, H], bf16)
        nc.sync.dma_start(bv_bf[:], d_bv[:])
        bo_bf = const_p.tile([1, H], bf16)
        nc.sync.dma_start(bo_bf[:], d_bo[:])
        gamma_r = const_p.tile([1, H], bf16)
        nc.sync.dma_start(gamma_r[:], d_gamma[:])
        beta_r = const_p.tile([1, H], bf16)
        nc.sync.dma_start(beta_r[:], d_beta[:])

        # long-lived activations (kT/qT: 64-partition tiles, base-0 reads)
        kT = stat_p.tile([DH, NH, S], bf16)
        v_sb = stat_p.tile([P, SC, NH, DH + 1], bf16)
        qT = stat_p.tile([DH, NH, I_CORE], bf16)
        qPair8 = stat_p.tile([P, NPAIR, 32], f8)    # block-diag bias weights
        nc.vector.memset(qPair8[:], 0.0)
        hidR = stat_p.tile([P, 2, H], f32)
        nc.sync.dma_start(hidR[:], d_hidR[:].transpose([1, 0, 2]))
        WoT = stat_p.tile([P, HC, H], bf16)
        nc.scalar.dma_start(WoT[:], d_WoT[:].transpose([1, 0, 2]))
        gammaB = stat_p.tile([P, H], bf16)
        betaB = stat_p.tile([P, H], bf16)
        ctxT = stat_p.tile([P, HC, IH], bf16)

        # bpe granule streaming
        bpe_tiles = {}

        def fetch_gran(g):
            t = bpe_p.tile([P, NGRAN, S], f8, name="bpeg")
            eng = nc.sync if g % 2 == 0 else nc.scalar
            eng.dma_start(t[:], d_bpe[:, g * NGRAN:(g + 1) * NGRAN, :])
            bpe_tiles[g] = t
            return t

        # ---------------- phase P: projections ----------------
        with tc.tile_pool(name="w8", bufs=1) as w8_p, \
             tc.tile_pool(name="psP", bufs=3, space=bass.MemorySpace.PSUM) \
                as psP, \
             tc.tile_pool(name="psG", bufs=2, space=bass.MemorySpace.PSUM) \
                as psG:
            Wq8 = w8_p.tile([P, HC, H], f8)
            nc.sync.dma_start(Wq8[:], d_Wq8[:].transpose([1, 0, 2]))
            hidRT8 = w8_p.tile([P, HC, I_CORE], f8)
            nc.scalar.dma_start(hidRT8[:], d_hidRT8[:].transpose([1, 0, 2]))
            WkT = w8_p.tile([P, HC, H], bf16)
            nc.scalar.dma_start(WkT[:], d_Wk[:].transpose([1, 0, 2]))
            hidT = w8_p.tile([P, HC, S], bf16)
            nc.sync.dma_start(hidT[:], d_hidT[:].transpose([1, 0, 2]))
            WvT = w8_p.tile([P, HC, H], bf16)
            nc.scalar.dma_start(WvT[:], d_Wv[:].transpose([1, 0, 2]))

            fetch_gran(0)
            fetch_gran(1)

            # Q projection (transposed): psum = 16*(Wq @ hidR^T)
            for r in range(HC):
                pq = psP.tile([P, 512], f32, name="pp")
                for kc in range(HC):
                    nc.tensor.matmul(pq[:, 0:I_CORE],
                                     Wq8[:, kc, r * P:(r + 1) * P],
                                     hidRT8[:, kc, :],
                                     start=(kc == 0), stop=(kc == HC - 1))
                for sub in range(2):
                    n = 2 * r + sub
                    srcp = pq[sub * DH:(sub + 1) * DH, 0:I_CORE]
                    bq_s = bqcol[sub * DH:(sub + 1) * DH, r:r + 1]
                    nc.vector.tensor_scalar(qT[:, n, :], srcp, 1.0 / WSCALE,
                                            bq_s, Alu.mult, Alu.add)
                    bq4_s = bqcol4[sub * DH:(sub + 1) * DH, r:r + 1]
                    nc.scalar.activation(
                        qPair8[sub * DH:(sub + 1) * DH, :, 2 * n:2 * n + 2],
                        srcp.rearrange("p (a b) -> p a b", b=2),
                        Act.Identity, scale=4.0 / WSCALE, bias=bq4_s)

            # K projection (transposed): kT = Wk @ hid^T + bk
            for r in range(HC):
                for jh in range(2):
                    pk = psP.tile([P, 512], f32, name="pp")
                    for kc in range(HC):
                        nc.tensor.matmul(pk[:], WkT[:, kc, r * P:(r + 1) * P],
                                         hidT[:, kc, jh * 512:(jh + 1) * 512],
                                         start=(kc == 0), stop=(kc == HC - 1))
                    for sub in range(2):
                        nc.vector.tensor_scalar(
                            kT[:, 2 * r + sub, jh * 512:(jh + 1) * 512],
                            pk[sub * DH:(sub + 1) * DH, :],
                            bkcol[sub * DH:(sub + 1) * DH, r:r + 1], None,
                            Alu.add)

            # V projection (natural): v = hid @ Wv^T + bv, + ones column
            for jc in range(SC):
                for vh in range(2):
                    pv = psP.tile([P, 512], f32, name="pp")
                    for kc in range(HC):
                        nc.tensor.matmul(pv[:, 0:VH],
                                         hidT[:, kc, jc * P:(jc + 1) * P],
                                         WvT[:, kc, vh * VH:(vh + 1) * VH],
                                         start=(kc == 0), stop=False)
                    nc.tensor.matmul(pv[:, 0:VH], ones_row[:],
                                     bv_bf[:, vh * VH:(vh + 1) * VH],
                                     start=False, stop=True)
                    nc.vector.tensor_copy(
                        v_sb[:, jc, vh * HP:(vh + 1) * HP, 0:DH],
                        pv[:, 0:VH].rearrange("p (a b) -> p a b", a=HP))
            nc.vector.memset(v_sb[:, :, :, DH], 1.0)

            # gamma/beta broadcast via K=1 matmuls (own pool, end of phase)
            for c in range(HC):
                pbx = psG.tile([P, P], f32, name="pbx")
                nc.tensor.matmul(pbx[:], ones_row[:],
                                 gamma_r[:, c * P:(c + 1) * P])
                nc.scalar.copy(gammaB[:, c * P:(c + 1) * P], pbx[:])
                pbx2 = psG.tile([P, P], f32, name="pbx")
                nc.tensor.matmul(pbx2[:], ones_row[:],
                                 beta_r[:, c * P:(c + 1) * P])
                nc.scalar.copy(betaB[:, c * P:(c + 1) * P], pbx2[:])

        # ---------------- per-half phases ----------------
        for h in range(2):
            # ---- phase A: bias einsum + permuted transpose ----
            with tc.tile_pool(name="psB", bufs=2, space=bass.MemorySpace.PSUM) \
                    as psB, \
                 tc.tile_pool(name="psT", bufs=2, space=bass.MemorySpace.PSUM) \
                    as psT:
                biasT = biasT_p.tile([P, SC, NH, NOCT, 8], bf16, name="biasT")
                for g in range(8):
                    gg = h * 8 + g
                    if gg not in bpe_tiles:
                        fetch_gran(gg)
                    gt = bpe_tiles[gg]
                    if gg + 2 <= 15 and (gg + 2) not in bpe_tiles:
                        fetch_gran(gg + 2)
                    for o2 in range(2):
                        oct_ = g * 2 + o2
                        pb = psB.tile([P, 1024], f32, name="pb")
                        for c4 in range(4):
                            pr = h * (NPAIR // 2) + oct_ * 4 + c4
                            wi = pr % NGRAN
                            for jh in range(2):
                                nc.tensor.matmul(
                                    pb[32 * c4:32 * c4 + 32,
                                       jh * 512:(jh + 1) * 512],
                                    qPair8[:, pr, :],
                                    gt[:, wi, jh * 512:(jh + 1) * 512],
                                    tile_position=(0, 32 * c4))
                        b4t = b4_p.tile([P, S], bf16, name="b4")
                        nc.scalar.activation(b4t[:], pb[:], Act.Copy,
                                             scale=0.25)
                        ptb = psT.tile([P, SC, P], bf16, name="ptb")
                        for jc in range(SC):
                            nc.tensor.transpose(ptb[:, jc, :],
                                                b4t[:, jc * P:(jc + 1) * P],
                                                perm_bf[:])
                        nc.vector.tensor_copy(
                            biasT[:, :, :, oct_, :],
                            ptb[:, :, 0:96].rearrange(
                                "p a (b c) -> p a b c", b=NH))

            # ---- phase B: attention (QK + bias + softmax + PV fused) ----
            with tc.tile_pool(name="psS", bufs=2, space=bass.MemorySpace.PSUM) \
                    as psS, \
                 tc.tile_pool(name="psC", bufs=1, space=bass.MemorySpace.PSUM) \
                    as psC:
                pctx = psC.tile([DH + 1, NH, IH], f32, name="pctx")
                for jc in range(SC):
                    for g2 in range(2):
                        n0 = g2 * HP
                        ps_s = psS.tile([P, HP, IH], f32, name="scores")
                        for nn in range(HP):
                            n = n0 + nn
                            nc.tensor.matmul(
                                ps_s[:, nn, :],
                                kT[:, n, jc * P:(jc + 1) * P],
                                qT[:, n, h * IH:(h + 1) * IH])
                        flat = ps_s[:].rearrange("p a b -> p (a b)")
                        nc.vector.tensor_tensor(
                            flat, flat,
                            biasT[:, jc, n0:n0 + HP].rearrange(
                                "p a b c -> p (a b c)"), Alu.add)
                        pt = probs_p.tile([P, HP, IH], bf16, name="probsT")
                        nc.scalar.activation(
                            pt[:].rearrange("p a b -> p (a b)"), flat,
                            Act.Exp, scale=0.125)
                        for nn in range(HP):
                            n = n0 + nn
                            nc.tensor.matmul(pctx[:, n, :],
                                             v_sb[:, jc, n, :],
                                             pt[:, nn, :],
                                             start=(jc == 0),
                                             stop=(jc == SC - 1),
                                             skip_group_check=True)

                sumsB = y_p.tile([1, NH, IH], bf16, name="sumsB")
                nc.vector.tensor_copy(sumsB[0:1, :, :], pctx[DH:DH + 1, :, :])
                for g2 in range(2):
                    prs = psS.tile([DH, HP, IH], f32, name="scores")
                    for nn in range(HP):
                        nc.tensor.matmul(prs[:, nn, :], ones_row[:, 0:DH],
                                         sumsB[0:1, g2 * HP + nn, :])
                    lnS = y_p.tile([DH, HP, IH], f32, name="lnS")
                    nc.scalar.activation(
                        lnS[:].rearrange("p a b -> p (a b)"),
                        prs[:].rearrange("p a b -> p (a b)"), Act.Ln)
                    precS = y_p.tile([DH, HP, IH], f32, name="precS")
                    nc.scalar.activation(
                        precS[:].rearrange("p a b -> p (a b)"),
                        lnS[:].rearrange("p a b -> p (a b)"), Act.Exp,
                        scale=-1.0)
                    for sub in range(2):
                        nc.vector.tensor_tensor(
                            ctxT[sub * DH:(sub + 1) * DH,
                                 3 * g2:3 * g2 + 3, :],
                            pctx[0:DH, g2 * HP + sub:g2 * HP + HP:2, :],
                            precS[:, sub:HP:2, :], Alu.mult)

                # ---- O proj + residual + LN ----
                y = y_p.tile([P, H], f32, name="yy")
                for vh in range(2):
                    # shares the "scores" slot rotation to stay in 8 banks
                    py = psS.tile([P, VH], f32, name="scores")
                    for kc in range(HC):
                        nc.tensor.matmul(py[:], ctxT[:, kc, :],
                                         WoT[:, kc, vh * VH:(vh + 1) * VH],
                                         start=(kc == 0), stop=False)
                    nc.tensor.matmul(py[:], ones_row[:],
                                     bo_bf[:, vh * VH:(vh + 1) * VH],
                                     start=False, stop=True)
                    nc.vector.tensor_tensor(y[:, vh * VH:(vh + 1) * VH],
                                            py[:],
                                            hidR[:, h, vh * VH:(vh + 1) * VH],
                                            Alu.add)
                mu = y_p.tile([P, 1], f32, name="mu")
                nc.vector.tensor_reduce(mu[:], y[:], AxisX, Alu.add)
                nc.vector.tensor_scalar(mu[:], mu[:], 1.0 / H, None, Alu.mult)
                yc = y_p.tile([P, H], f32, name="yc")
                nc.vector.tensor_scalar(yc[:], y[:], mu[:], None, Alu.subtract)
                ssq = y_p.tile([P, 1], f32, name="ssq")
                nc.scalar.activation(y[:], yc[:], Act.Square, accum_out=ssq[:])
                std = y_p.tile([P, 1], f32, name="std")
                nc.scalar.activation(std[:], ssq[:], Act.Sqrt,
                                     scale=1.0 / H, bias=eps_t[:])
                rstd = y_p.tile([P, 1], f32, name="rstd")
                nc.vector.reciprocal(rstd[:], std[:])
                o1 = y_p.tile([P, H], f32, name="o1")
                nc.vector.tensor_scalar(o1[:], yc[:], rstd[:], None, Alu.mult)
                nc.vector.tensor_tensor(o1[:], o1[:], gammaB[:], Alu.mult)
                nc.vector.tensor_tensor(o1[:], o1[:], betaB[:], Alu.add)
                nc.sync.dma_start(d_out[h], o1[:])

    nc.compile()
    return nc


def _build_perm():
    """Permutation: transpose output column f <- b4 strip row sigma(f).
    f-order: (n, c4, par) for f<96; sigma(f) = 32*c4 + 2*n + par."""
    perm = np.zeros((P, P), np.float32)
    for n in range(NH):
        for c4 in range(4):
            for par in range(2):
                f = n * 8 + c4 * 2 + par
                perm[32 * c4 + 2 * n + par, f] = 1.0
    for c4 in range(4):
        for u in range(8):
            perm[32 * c4 + 24 + u, 96 + c4 * 8 + u] = 1.0
    return perm


def _shard_inputs(inputs):
    import ml_dtypes
    bf = ml_dtypes.bfloat16
    f8 = ml_dtypes.float8_e3m4
    hs = np.ascontiguousarray(np.asarray(inputs["hidden_states"]), np.float32)
    bpe = np.asarray(inputs["bbox_pos_emb"])
    HC = H // P

    perm = _build_perm().astype(bf)
    Wq8 = np.ascontiguousarray(
        (np.asarray(inputs["Wq"], np.float32).T * WSCALE).astype(f8)).reshape(
            HC, P, H)
    WkT = np.ascontiguousarray(
        np.asarray(inputs["Wk"], np.float32).T.astype(bf)).reshape(HC, P, H)
    WvT = np.ascontiguousarray(
        np.asarray(inputs["Wv"], np.float32).T.astype(bf)).reshape(HC, P, H)
    WoT = np.ascontiguousarray(
        np.asarray(inputs["Wo"], np.float32).T.astype(bf)).reshape(HC, P, H)

    def col(name, scale=1.0):
        v = np.asarray(inputs[name], np.float32) * scale
        return np.ascontiguousarray(v.reshape(HC, P).T)  # [P, HC]

    bqcol = col("bq")
    bqcol4 = col("bq", 4.0)
    bkcol = col("bk")
    bv_bf = np.asarray(inputs["bv"], np.float32).astype(bf).reshape(1, H)
    bo_bf = np.asarray(inputs["bo"], np.float32).astype(bf).reshape(1, H)
    gamma_bf = np.asarray(inputs["ln_gamma"], np.float32).astype(bf).reshape(1, H)
    beta_bf = np.asarray(inputs["ln_beta"], np.float32).astype(bf).reshape(1, H)

    hsT = {b: np.ascontiguousarray(hs[b].T).astype(bf).reshape(HC, P, S)
           for b in range(B)}

    in_maps = []
    for c in range(N_CORES):
        b = c // 4
        q0 = (c % 4) * I_CORE
        # bpe8 [128, 128, 1024]: row p=(i%2)*64+d, [pair, j]
        x = bpe[q0:q0 + I_CORE, :, b, :]            # [256 i, 1024 j, 64 d]
        x = np.asarray(x, np.float32).reshape(I_CORE // 2, 2, S, DH)
        x = x.transpose(1, 3, 0, 2).reshape(P, I_CORE // 2, S)
        m = {
            "hidT_bf": hsT[b],
            "hidRT8": np.ascontiguousarray(
                hs[b, q0:q0 + I_CORE].T).astype(f8).reshape(HC, P, I_CORE),
            "hidR": np.ascontiguousarray(
                hs[b, q0:q0 + I_CORE].reshape(2, P, H)),
            "bpe8": np.ascontiguousarray(x).astype(f8),
            "WoT": WoT, "Wq8": Wq8, "WkT_bf": WkT, "WvT_bf": WvT,
            "bqcol": bqcol, "bqcol4": bqcol4, "bkcol": bkcol,
            "bv_bf": bv_bf, "bo_bf": bo_bf,
            "gamma_bf": gamma_bf, "beta_bf": beta_bf,
            "perm_bf": perm,
        }
        in_maps.append(m)
    return in_maps


def _install_ntff_shim():
    """The agent image's antenv lacks axon_hooks; recreate the NTFF profile
    hook via ctypes against libaxon_pjrt.so so trace=True yields
    exec_time_ns + a perfetto trace."""
    import sys as _sys
    if "antenv.axon_hooks" in _sys.modules:
        return
    import types, ctypes, contextlib
    so_path = "/opt/axon/libaxon_pjrt.so"
    mod = types.ModuleType("antenv.axon_hooks")
    _state = {}

    def get_axon_ntff_profile_hook():
        if "hook" in _state:
            return _state["hook"]
        try:
            lib = ctypes.CDLL(so_path)
            if not hasattr(lib, "axon_start_nrt_profile"):
                _state["hook"] = None
                return None
            lib.axon_start_nrt_profile.argtypes = [
                ctypes.POINTER(ctypes.c_int64), ctypes.c_size_t]
            lib.axon_start_nrt_profile.restype = ctypes.c_int64
            lib.axon_stop_nrt_profile.argtypes = [ctypes.c_char_p]
            lib.axon_stop_nrt_profile.restype = ctypes.c_int64
        except OSError:
            _state["hook"] = None
            return None

        @contextlib.contextmanager
        def _hook(output_dir, device_ids):
            import jax
            jax.devices()
            if device_ids:
                ids = (ctypes.c_int64 * len(device_ids))(*device_ids)
                rc = lib.axon_start_nrt_profile(ids, len(device_ids))
            else:
                rc = lib.axon_start_nrt_profile(None, 0)
            if rc != 0:
                raise RuntimeError(f"axon_start_nrt_profile rc={rc}")
            try:
                yield
            finally:
                n = lib.axon_stop_nrt_profile(str(output_dir).encode())
                print(f"ntff profile: {n} file(s) written to {output_dir}")

        _state["hook"] = _hook
        return _hook

    mod.get_axon_ntff_profile_hook = get_axon_ntff_profile_hook
    _sys.modules["antenv.axon_hooks"] = mod


def kernel(**inputs):
    from concourse.bass_utils import run_bass_kernel_spmd

    if os.environ.get("BASS_KERNEL_TRACE"):
        _install_ntff_shim()
        import concourse.bass_utils as _bu
        _bu.upload_artifacts = lambda tmpdir: f"file://{tmpdir}"

    if "nc" not in _COMPILED:
        _COMPILED["nc"] = build_kernel()
    nc = _COMPILED["nc"]
    in_maps = _shard_inputs(inputs)
    res = run_bass_kernel_spmd(nc, in_maps, core_ids=list(range(N_CORES)),
                               trace=bool(os.environ.get("BASS_KERNEL_TRACE")))
    _COMPILED["last_result"] = res
    out = np.zeros((B, S, H), dtype=np.float32)
    for c in range(N_CORES):
        b = c // 4
        q0 = (c % 4) * I_CORE
        out[b, q0:q0 + I_CORE] = np.asarray(
            res.results[c]["out"]).reshape(I_CORE, H)
    return out
